# revision 1
# baseline (speedup 1.0000x reference)
"""Trainium2 Bass kernel (v8) for LocalDynamicGraph edge-feature construction.

Per batch element b (one NeuronCore each, data-parallel over B=8):
    out[b, n, c, k] = x[b, idx[b,n,k], c] - x[b, n, c]   for c < 64
    out[b, n, c, k] = x[b, n, c - 64]                    for c >= 64

v5 pipeline (per core):
  - SWDGE gather desc-gen on the Pool engine is the serial bottleneck
    (~2.25ns/index + ~1us/call fixed; queues do NOT parallelize it), so
    steady-state calls carry 2048 indices = one full point-block. The
    gathered payload is bf16 (128B/row, 129-desc/16.5KB packets — the
    same packet bytes as the known-good fp32/1024 config) to halve SDMA
    engine time and HBM gather-read traffic.
  - The first WARMB blocks use fp32 1024-index calls straight from x so
    desc-gen starts immediately instead of stalling ~35us behind the
    bf16 staging round-trip (x -> SBUF -> cast -> padded bf16 HBM).
  - DVE computes (neighbor - center) into the (c, k)-interleaved first
    half of the output tile; ACT broadcast-copies the fp32 center into
    the second half (bit-exact; bf16 rounding only perturbs the
    difference term, ~0.1% rel L2 vs the 2e-2 gate).
  - HWDGE writes each finished (128 points, 2048 ch*k) tile back as one
    fully contiguous 1MB DMA.
"""

import numpy as np

import concourse.bacc as bacc
import concourse.mybir as mybir
from concourse.tile import TileContext
from concourse.bass_utils import run_bass_kernel_spmd

# Problem constants (hardcoded per contest contract).
B = 8
N = 8192
C = 64
K = 16
P = 128              # partitions / points per output tile
NBLK = N // P        # 64 point-blocks per core
NQ = 4               # SWDGE queues
WARMB = 12           # leading point-blocks gathered fp32 from x
GW = 1024            # indices per warm call (2 calls per warm block)
GS = 2048            # indices per steady call (1 call per block)
XPAD = 128           # bf16 row padded to 128 elements = 256B stride
# idxw column layout: warm blocks first (2 calls x 64 cols each), then
# steady blocks (1 call x 128 cols). Total columns = NBLK*K*P/16 = 8192.
COLS = NBLK * K * P // 16

_NC_CACHE = {}


def _dma_gather_raw(gp, out_ap, in_ap, idxs_ap, num_idxs, num_idxs_reg,
                    elem_size, elem_step, queue_num, single_packet=True):
    """bass.dma_gather minus the elem_size%256B assert. The SWDGE ucode
    only needs the source stride (elem_step bytes) to be a multiple of
    256; the per-index payload is a plain descriptor length. Verified
    against q7_kernels/extended_inst/dma_gather.cpp."""
    dtsize = mybir.dt.size(in_ap.dtype)
    assert in_ap.dtype == out_ap.dtype
    assert idxs_ap.dtype == mybir.dt.int16
    stride_bytes = elem_step * dtsize
    assert stride_bytes % 256 == 0
    stride_256 = stride_bytes // 256
    assert 0 < stride_256 < 256
    assert in_ap.ap[0][0] == elem_step
    assert in_ap.ap[-1][1] == elem_size
    assert out_ap.ap[-1][1] == elem_size
    assert out_ap.ap[0][1] * out_ap.ap[1][1] == num_idxs
    _in_ap = gp.lower_ap_dma(in_ap, for_custom_bir_dma=True)
    _idxs_ap = gp.lower_ap(idxs_ap)
    _out_ap = gp.lower_ap(out_ap)
    return gp.add_instruction(
        mybir.InstDMAGatherAnt(
            name=gp.bass.get_next_instruction_name(),
            ins=[
                *_in_ap,
                _idxs_ap,
                gp.lower_val_access(gp.to_reg(num_idxs_reg)),
            ],
            outs=[_out_ap],
            transpose=False,
            num_idxs=num_idxs,
            elem_size=elem_size,
            stride_bytes_256=stride_256,
            gen_mode=0,
            single_packet=single_packet,
            queue_num=queue_num,
            sbuf_tokens_per_rank=0,
            sbuf_free_dim_per_rank=0,
            sbuf_free_dim_pad_per_rank=0,
            sbuf_byte_offset=0,
        )
    )


def build_nc():
    nc = bacc.Bacc(
        "TRN2",
        target_bir_lowering=False,
        dynamic_dma_scratch_size=32768,
        num_swdge_queues=NQ,
    )
    x = nc.dram_tensor("x", [N, C], mybir.dt.float32, kind="ExternalInput").ap()
    idxw = nc.dram_tensor(
        "idxw", [P, COLS], mybir.dt.int16, kind="ExternalInput"
    ).ap()
    xbf = nc.dram_tensor("xbf", [N, XPAD], mybir.dt.bfloat16, kind="Internal").ap()
    out = nc.dram_tensor(
        "out", [N, 2 * C * K], mybir.dt.float32, kind="ExternalOutput"
    ).ap()
    out_blocks = out.rearrange("(nb p) f -> nb p f", p=P)

    with TileContext(nc) as tc:
        with (
            tc.tile_pool(name="const", bufs=1) as const_pool,
            tc.tile_pool(name="gat", bufs=8) as gpool,
            tc.tile_pool(name="outp", bufs=6) as opool,
        ):
            # Wrapped indices, replicated across all 16-partition groups
            # (the gather ucode reads each queue's own 32-partition band;
            # the simulator reads partitions 0-15 — replication satisfies
            # both). Loaded in chunks so early gathers aren't gated on the
            # full 2MB transfer.
            idx_sb = const_pool.tile([P, COLS], mybir.dt.int16)
            IDX_CHUNKS = 16
            ccols = COLS // IDX_CHUNKS
            for ch in range(IDX_CHUNKS):
                nc.sync.dma_start(
                    idx_sb[:, ch * ccols : (ch + 1) * ccols],
                    idxw[:, ch * ccols : (ch + 1) * ccols],
                )
            # Whole x staged in SBUF: partition p, free (nb, c) = x[nb*128+p, c]
            xall = const_pool.tile([P, NBLK * C], mybir.dt.float32)
            nc.sync.dma_start(
                xall[:].rearrange("p (nb c) -> p nb c", c=C),
                x.rearrange("(nb p) c -> p nb c", p=P),
            )
            # bf16 copy of x, rows padded to 256B: write the row twice so
            # every padded byte is initialized without a separate memset.
            xbf_sb = const_pool.tile([P, NBLK * XPAD], mybir.dt.bfloat16)
            nc.vector.tensor_copy(
                xbf_sb[:].rearrange("p (nb r c) -> p nb r c", r=2, c=C),
                xall[:]
                .rearrange("p (nb c) -> p nb c", c=C)
                .unsqueeze(2)
                .broadcast_to([P, NBLK, 2, C]),
            )
            # Tile's shadow-memory dep tracking orders the gather reads of
            # xbf (DRAM) after this write completes.
            nc.sync.dma_start(
                xbf.rearrange("(nb p) c -> p nb c", p=P),
                xbf_sb[:].rearrange("p (nb c) -> p nb c", c=XPAD),
            )

            xbf_src = xbf[:, 0:C]  # ap [(XPAD, N), (1, C)]: 256B stride, 128B payload
            nwarm_reg = nc.gpsimd.to_reg(GW)
            nsteady_reg = nc.gpsimd.to_reg(GS)
            qi = 0   # global call counter -> SWDGE queue
            col = 0  # running idx_sb column offset
            for nb in range(NBLK):
                warm = nb < WARMB
                gdt = mybir.dt.float32 if warm else mybir.dt.bfloat16
                gt = gpool.tile([P, K * C], gdt)
                calls = (
                    [(GW, 0), (GW, 1)] if warm else [(GS, 0)]
                )
                for gc, half in calls:
                    grows = gc // P
                    _dma_gather_raw(
                        nc.gpsimd,
                        out_ap=gt[
                            :, half * grows * C : (half + 1) * grows * C
                        ].rearrange("p (g c) -> p g c", c=C),
                        in_ap=x if warm else xbf_src,
                        idxs_ap=idx_sb[:, col : col + gc // 16],
                        num_idxs=gc,
                        num_idxs_reg=nwarm_reg if warm else nsteady_reg,
                        elem_size=C,
                        elem_step=C if warm else XPAD,
                        queue_num=qi % NQ,
                        # >64-desc concatenated packets hang the SDMA; the
                        # 2048-index steady calls use per-descriptor packets.
                        single_packet=warm,
                    )
                    qi += 1
                    col += gc // 16
                ot = opool.tile([P, 2 * C * K], mybir.dt.float32)
                neigh = (
                    gt[:].rearrange("p (r c) -> p r c", c=C).transpose([0, 2, 1])
                )  # (P, C, K) strided view of the k-major gathered rows
                centr = xall[:, nb * C : (nb + 1) * C]  # (P, C)
                centr_b = centr.unsqueeze(2).broadcast_to([P, C, K])
                dst1 = ot[:, 0 : C * K].rearrange("p (c k) -> p c k", k=K)
                dst2 = ot[:, C * K : 2 * C * K].rearrange("p (c k) -> p c k", k=K)
                nc.vector.tensor_sub(dst1, neigh, centr_b)
                nc.scalar.copy(dst2, centr_b)
                nc.sync.dma_start(out_blocks[nb], ot[:])
            assert col == COLS, col
    nc.compile()
    return nc


def get_nc():
    if "nc" not in _NC_CACHE:
        _NC_CACHE["nc"] = build_nc()
    return _NC_CACHE["nc"]


def _prep_indices(idx: np.ndarray) -> np.ndarray:
    """int (B, N, K) neighbor indices -> wrapped int16 (B, 128, COLS)
    SWDGE gather index tensors (per core).

    Flat instance order per block is (k, p): logical position
    l = k*128 + p holds idx[nb*128 + p, k], so gathered row l lands in
    partition l%128 at free slot l//128 = k. Warm blocks are split into
    two 1024-index calls, steady blocks are one 2048-index call; each
    call's indices are wrapped (l%16 -> partition row, l//16 -> column)
    and the calls' column ranges are concatenated in issue order,
    replicated across all eight 16-partition GPSIMD core groups."""
    idx16 = idx.astype(np.int16)  # (B, N, K)
    arr = idx16.reshape(B, NBLK, P, K)
    flat = arr.transpose(0, 1, 3, 2).reshape(B, NBLK, K * P)  # (b, nb, l)
    cols = []
    for nb in range(NBLK):
        sizes = [GW, GW] if nb < WARMB else [GS]
        off = 0
        for gc in sizes:
            call = flat[:, nb, off : off + gc]  # (B, gc)
            cols.append(call.reshape(B, gc // 16, 16).transpose(0, 2, 1))
            off += gc
    wrapped = np.concatenate(cols, axis=2)  # (B, 16, COLS)
    rep = np.broadcast_to(wrapped[:, None, :, :], (B, 8, 16, COLS))
    idxw = rep.reshape(B, P, COLS)
    return np.ascontiguousarray(idxw)


def run_on_hw(x: np.ndarray, idx: np.ndarray, **spmd_kwargs):
    """Run the bass kernel on 8 NeuronCores. Returns (out, BassKernelResults)."""
    x = np.ascontiguousarray(np.asarray(x, dtype=np.float32))
    idx = np.asarray(idx)
    idxw = _prep_indices(idx)
    in_maps = [{"x": x[b], "idxw": idxw[b]} for b in range(B)]
    res = run_bass_kernel_spmd(get_nc(), in_maps, core_ids=list(range(B)), **spmd_kwargs)
    out = np.stack([r["out"].reshape(N, 2 * C, K) for r in res.results])
    return out, res


def kernel(x: np.ndarray, idx: np.ndarray) -> np.ndarray:
    out, _ = run_on_hw(x, idx)
    return out

